# revision 10
# baseline (speedup 1.0000x reference)
"""Trainium2 Bass kernel for nn_Block2DGRU (norm->dwconv3x3->bi-minGRU->norm->MLP).

fp8e4m3 DoubleRow matmuls for the heavy GEMMs (GRU in-proj, MLP p1/p2, conv),
bf16 for the GRU out-projection (scan-output fp8 quantization dominates the
error budget), f32r for layernorm stats/broadcast matmuls.

Layout: [feature_on_partitions, time_on_free], 8 NeuronCores SPMD, 2 batch
items per core.  Per batch item:
  - x_t:   3x[128, L] f32 input chunks; overwritten in place by the GRU
           residual accumulation (y = gru1 + gru2 + x).
  - xh:    fp8 normalized input, 4 K-planes (K padded 384->512 for DoubleRow),
           row pitch 57 (zero pad column) so conv column shifts are plain +-1
           element offsets; 1-element zero guard in front.
  - hc:    [128, 4, L] fp8 conv output (contiguous, plane 3 zero).
  - hs:    [128, 6, L] bf16 scan outputs per GRU.
  - yh:    fp8 norm2 output stored in the xh slot (contiguous plane slices).
Gate-half GRU weights are pre-negated so sigmoid() yields a = 1-z directly;
b = z*g = g - a*g is built on GpSimd.
"""
import numpy as np
import ml_dtypes

import concourse.bass as bass
import concourse.tile as tile
import concourse.mybir as mybir
from concourse.bass_utils import run_bass_kernel_spmd

F32 = mybir.dt.float32
F32R = mybir.dt.float32r
BF16 = mybir.dt.bfloat16
FP8 = mybir.dt.float8e4
AF = mybir.ActivationFunctionType
ALU = mybir.AluOpType
DRM = mybir.MatmulPerfMode.DoubleRow

NB = 56
NBP = 57                     # padded row pitch
L = NB * NB                  # 3136
LP = NBP * NB                # 3192
D = 384
DC = 3
DI = 768
MLPD = 1536
B = 2
NCORES = 8
NT = 448                     # time block = 8 image rows
NBLK = 7
SUB = 224                    # DoubleRow moving sub-block
NTP = 8 * NBP                # 456: padded conv psum block
HALVES = [(0, 1792, (0, 1, 2, 3)), (1792, 1344, (4, 5, 6))]
STATS_GROUPS = [(0, 1, 2), (3, 4, 5), (6,)]
EPS = 1e-5
WS = 32.0                    # fp8 scale for p1/p2 weights (+WS*p2b fold)
CONVS = 16.0                 # conv weight scale


def _fix_multiwaits(nc):
    """The walrus accepts at most ONE sync wait per instruction; hoist
    extras into wait-only NoOps on the same engine (streams are in-order)."""
    n = 0
    cnt = [0]
    for f in nc.m.functions:
        for bb in f.blocks:
            out = []
            for inst in bb.instructions:
                si = inst.sync_info
                if si is not None and si.on_wait is not None and len(si.on_wait) > 1:
                    waits = list(si.on_wait)
                    for w in waits[:-1]:
                        cnt[0] += 1
                        nop = mybir.InstNoOp(
                            name=f"I-waitfix-{cnt[0]}",
                            sync_info=mybir.SyncInfo(on_wait=[w], on_update=[]),
                        )
                        nop.engine = inst.engine
                        out.append(nop)
                    inst.sync_info = mybir.SyncInfo(
                        on_wait=[waits[-1]], on_update=list(si.on_update or [])
                    )
                    n += 1
                out.append(inst)
            bb.instructions = out
    return n


def build_kernel():
    nc = bass.Bass("TRN2", target_bir_lowering=False, debug=False,
                   num_devices=NCORES)

    xT_d = nc.dram_tensor("xT", [B, D, L], F32, kind="ExternalInput").ap()
    whg_d = nc.dram_tensor("whg8", [2, 128, 3, 1536], FP8,
                           kind="ExternalInput").ap()
    wout_d = nc.dram_tensor("wout16", [2, 128, 6, D], BF16,
                            kind="ExternalInput").ap()
    p1_d = nc.dram_tensor("p18", [128, 3, MLPD], FP8, kind="ExternalInput").ap()
    p2_d = nc.dram_tensor("p28", [128, 12, D], FP8, kind="ExternalInput").ap()
    cw_d = nc.dram_tensor("convw8", [DC, 128, 9, 128], FP8,
                          kind="ExternalInput").ap()
    dwb_d = nc.dram_tensor("dwb", [128, DC], F32, kind="ExternalInput").ap()
    p1b_d = nc.dram_tensor("p1b", [128, 12], F32, kind="ExternalInput").ap()
    p2bT_d = nc.dram_tensor("p2bT", [1, D], BF16, kind="ExternalInput").ap()
    ident_d = nc.dram_tensor("ident", [128, 128], BF16,
                             kind="ExternalInput").ap()
    out_d = nc.dram_tensor("outT", [B, D, L], F32, kind="ExternalOutput").ap()

    from contextlib import ExitStack
    with tile.TileContext(nc) as tc, ExitStack() as ctx:
        big = ctx.enter_context(tc.tile_pool(name="big", bufs=1))
        wpool = ctx.enter_context(tc.tile_pool(name="wpool", bufs=1))
        work = ctx.enter_context(tc.tile_pool(name="work", bufs=2))
        psum = ctx.enter_context(tc.tile_pool(name="psum", bufs=1, space="PSUM"))

        # ---------------- persistent weights / constants (loaded once)
        whg_t = [wpool.tile([128, 3, 1536], FP8, tag=f"whg{g}", name=f"whg{g}")
                 for g in range(2)]
        wout_t = [wpool.tile([128, 6, D], BF16, tag=f"wo{g}", name=f"wo{g}")
                  for g in range(2)]
        for g in range(2):
            nc.sync.dma_start(whg_t[g][:], whg_d[g])
            nc.sync.dma_start(wout_t[g][:], wout_d[g])
        p1w = wpool.tile([128, 3, MLPD], FP8, tag="p1w", name="p1w")
        nc.sync.dma_start(p1w[:], p1_d)
        p2w = wpool.tile([128, 12, D], FP8, tag="p2w", name="p2w")
        nc.sync.dma_start(p2w[:], p2_d)
        cw_t = [wpool.tile([128, 9, 128], FP8, tag=f"cw{c}", name=f"cw{c}")
                for c in range(DC)]
        for c in range(DC):
            nc.sync.dma_start(cw_t[c][:], cw_d[c])
        dwb_t = wpool.tile([128, DC], F32, tag="dwb", name="dwb")
        nc.sync.dma_start(dwb_t[:], dwb_d)
        p1b_t = wpool.tile([128, 12], F32, tag="p1b", name="p1b")
        nc.sync.dma_start(p1b_t[:], p1b_d)
        p2bT_t = wpool.tile([1, D], BF16, tag="p2bT", name="p2bT")
        nc.sync.dma_start(p2bT_t[:], p2bT_d)
        ident_t = wpool.tile([128, 128], BF16, tag="ident", name="ident")
        nc.sync.dma_start(ident_t[:], ident_d)

        zsel16 = wpool.tile([128, 129], BF16, tag="zsel16", name="zsel16")
        nc.vector.memset(zsel16[:], 0.0)
        nc.vector.memset(zsel16[:, 64:65], 1.0)
        ones1 = wpool.tile([65, 128], BF16, tag="ones1", name="ones1")
        nc.vector.memset(ones1[:], 1.0)
        onesN = wpool.tile([1, NT], BF16, tag="onesN", name="onesN")
        nc.vector.memset(onesN[:], 1.0)
        eps_t = wpool.tile([128, 1], F32, tag="eps", name="eps")
        nc.vector.memset(eps_t[:], EPS)

        def r32(ap):
            return ap.bitcast(F32R)

        for b in range(B):
            # ---------------- batch-persistent tiles
            x_t = [big.tile([128, L], BF16, tag=f"x{c}", name=f"x{c}",
                            bufs=2) for c in range(DC)]
            for c in range(DC):
                nc.gpsimd.dma_start(x_t[c][:],
                                    xT_d[b, c * 128:(c + 1) * 128, :])
            # padded fp8 activations: 1 guard element + 4 planes of LP
            xh = big.tile([128, 1 + 3 * LP], FP8, tag="act8", name="act8")
            xf = xh[:]
            nc.gpsimd.memset(xh[:, 0:1], 0.0)              # guard
            # zero the pad column (index 56 of each 57-row) of planes 0..2
            pads = bass.AP(tensor=xf.tensor, offset=xf.offset + 1 + NB,
                           ap=[list(xf.ap[0]), [LP, 3], [NBP, NB]])
            nc.gpsimd.memset(pads, 0.0)

            def xh_pl(c):
                return xh[:, 1 + c * LP: 1 + (c + 1) * LP]

            hc = big.tile([128, 3, L], FP8, tag="hc", name="hc")

            # ============================================= layernorm helper
            def norm_stats(src):
                """src(c, sl) -> f32 AP.  Returns [(numu, inv, blks), ...]
                with per-block stats rows at partitions {0,32,64,96}."""
                res = []
                for grp, blks in enumerate(STATS_GROUPS):
                    sA = psum.tile([65, NT], F32, tag="P0", name="sA")
                    sB = psum.tile([65, NT], F32, tag="P1", name="sB")
                    nlast = len(blks) - 1
                    for i, blk in enumerate(blks):
                        sl = slice(blk * NT, (blk + 1) * NT)
                        sq = work.tile([128, NT], BF16, tag="sq", name="sq",
                                       bufs=1)
                        for c in range(DC):
                            nc.scalar.activation(sq[:], src(c, sl), AF.Square)
                            nc.tensor.matmul(
                                sA[:], zsel16[:, 64 - 32 * i:129 - 32 * i],
                                src(c, sl), start=(i == 0 and c == 0),
                                stop=(i == nlast and c == DC - 1),
                                skip_group_check=True)
                            nc.tensor.matmul(
                                sB[:], zsel16[:, 64 - 32 * i:129 - 32 * i],
                                sq[:], start=(i == 0 and c == 0),
                                stop=(i == nlast and c == DC - 1),
                                skip_group_check=True)
                    numu = work.tile([65, NT], BF16, tag=f"numu{grp}",
                                     name=f"numu{grp}", bufs=1)
                    nc.scalar.activation(numu[:], sA[:], AF.Copy,
                                         scale=-1.0 / D)
                    scr = work.tile([65, NT], F32, tag="nscr", name="nscr",
                                    bufs=1)
                    nc.scalar.activation(scr[:], sA[:], AF.Square,
                                         scale=1.0 / D)
                    nc.vector.scalar_tensor_tensor(scr[:], sB[:], 1.0 / D,
                                                   scr[:], ALU.mult,
                                                   ALU.subtract)
                    nc.scalar.activation(scr[:], scr[:], AF.Sqrt,
                                         bias=eps_t[0:65, :])
                    scr2 = work.tile([65, NT], F32, tag="nscr2",
                                     name="nscr2", bufs=1)
                    nc.vector.reciprocal(scr2[:], scr[:])
                    inv = work.tile([65, NT], BF16, tag=f"inv{grp}",
                                    name=f"inv{grp}", bufs=1)
                    nc.vector.tensor_copy(inv[:], scr2[:])
                    res.append((numu, inv, blks))
                return res

            def norm_apply(stats, src, store):
                for numu, inv, blks in stats:
                    for i, blk in enumerate(blks):
                        sl = slice(blk * NT, (blk + 1) * NT)
                        ib = psum.tile([128, NT], F32, tag="P4", name="ib")
                        nc.tensor.matmul(ib[:],
                                         ones1[32 * i:32 * i + 1, :],
                                         inv[32 * i:32 * i + 1, :],
                                         start=True, stop=True)
                        ibs = work.tile([128, NT], BF16, tag="ibs",
                                        name="ibs", bufs=2)
                        nc.scalar.activation(ibs[:], ib[:], AF.Copy)
                        for c in range(DC):
                            apc = psum.tile([128, NT], F32,
                                            tag=f"P{2 + c % 2}", name="apc")
                            nc.tensor.matmul(apc[:],
                                             ones1[32 * i:32 * i + 1, :],
                                             numu[32 * i:32 * i + 1, :],
                                             start=True, stop=False,
                                             skip_group_check=True)
                            nc.tensor.matmul(apc[:], ident_t[:],
                                             src(c, sl),
                                             start=False, stop=True,
                                             skip_group_check=True)
                            store(c, blk, apc, ibs)

            # ------------------------------------------------ norm1 -> xh
            st1 = norm_stats(lambda c, sl: x_t[c][:, sl])

            def store_xh(c, blk, apc, ib):
                base = xh_pl(c)
                dst = bass.AP(tensor=base.tensor,
                              offset=base.offset + blk * 8 * NBP,
                              ap=[list(base.ap[0]), [NBP, 8], [1, NB]])
                src3 = apc[:].rearrange("p (rr cc) -> p rr cc", cc=NB)
                ib3 = ib[:].rearrange("p (rr cc) -> p rr cc", cc=NB)
                nc.vector.tensor_tensor(dst, src3, ib3, ALU.mult)

            norm_apply(st1, lambda c, sl: x_t[c][:, sl], store_xh)

            # ============================================= depthwise conv 3x3
            # cw slot order: [t4, t0,t2, t3,t5, t6,t8, t1,t7]
            for c in range(DC):
                base = xh_pl(c)
                pstride = list(base.ap[0])

                def mv(off, d1, n):
                    return bass.AP(tensor=base.tensor,
                                   offset=base.offset + off,
                                   ap=[pstride, [d1, 2], [1, n]])

                for slab in range(NBLK):
                    cp = psum.tile([128, NTP], F32,
                                   tag=("P6" if slab % 2 == 0 else "P7"),
                                   name="cp")
                    r0 = 8 * slab
                    # solo center tap (0,0): full padded range, starts group
                    for o0, o1 in ((0, 228), (228, NTP)):
                        nc.tensor.matmul(cp[:, o0:o1], cw_t[c][:, 0, :],
                                         base[:, r0 * NBP + o0:r0 * NBP + o1],
                                         start=(o0 == 0), stop=False,
                                         skip_group_check=True)
                    # column pairs (dr,-1)&(dr,+1): dim1 stride 2
                    for dr, slot in ((-1, 1), (0, 3), (1, 5)):
                        lo = max(0, -(r0 + dr))
                        hi = max(0, (r0 + 8 + dr) - NB)
                        orow0, orow1 = lo, 8 - hi
                        ncols = (orow1 - orow0) * NBP - 1
                        src0 = (r0 + orow0 + dr) * NBP - 1
                        mid = ncols // 2
                        for c0, c1 in ((0, mid), (mid, ncols)):
                            nc.tensor.matmul(
                                cp[:, orow0 * NBP + c0:orow0 * NBP + c1],
                                cw_t[c][:, slot:slot + 2, :],
                                mv(src0 + c0, 2, c1 - c0),
                                start=False, stop=False, perf_mode=DRM,
                                skip_group_check=True)
                    # row pair (-1,0)&(+1,0): dim1 stride 2*NBP
                    lo = max(0, 1 - r0)
                    hi = max(0, (r0 + 8) - (NB - 1))
                    orow0, orow1 = lo, 8 - hi
                    ncols = (orow1 - orow0) * NBP - 1
                    src0 = (r0 + orow0 - 1) * NBP
                    mid = ncols // 2
                    interior = slab not in (0, NBLK - 1)
                    for si, (c0, c1) in enumerate(((0, mid), (mid, ncols))):
                        nc.tensor.matmul(
                            cp[:, orow0 * NBP + c0:orow0 * NBP + c1],
                            cw_t[c][:, 7:9, :], mv(src0 + c0, 2 * NBP, c1 - c0),
                            start=False, stop=(interior and si == 1),
                            perf_mode=DRM, skip_group_check=True)
                    # slab edge rows 0 / 55 get only one of the (+-1,0) taps
                    if slab == 0:
                        nc.tensor.matmul(cp[:, 0:NB], cw_t[c][:, 8, :],
                                         base[:, NBP:NBP + NB],
                                         start=False, stop=True,
                                         skip_group_check=True)
                    if slab == NBLK - 1:
                        nc.tensor.matmul(cp[:, 7 * NBP:7 * NBP + NB],
                                         cw_t[c][:, 7, :],
                                         base[:, 54 * NBP:54 * NBP + NB],
                                         start=False, stop=True,
                                         skip_group_check=True)
                    # readout: drop pad columns, descale, +bias -> fp8
                    cin = cp[:].rearrange("p (rr cc) -> p rr cc",
                                          cc=NBP)[:, :, 0:NB]
                    hout = hc[:, c, slab * NT:(slab + 1) * NT].rearrange(
                        "p (rr cc) -> p rr cc", cc=NB)
                    nc.scalar.activation(hout, cin, AF.Identity,
                                         scale=1.0 / CONVS,
                                         bias=dwb_t[:, c:c + 1])

            # ============================================= bidirectional GRU
            hs_t = [big.tile([128, 6, L], BF16, tag=f"hs{g}", name=f"hs{g}")
                    for g in range(2)]
            pending_scan = [None]

            def flush_scan():
                if pending_scan[0] is not None:
                    for emit in pending_scan[0]:
                        emit()
                    pending_scan[0] = None

            for j in range(6):
                for g in range(2):
                    halves = HALVES if g == 0 else HALVES[::-1]
                    for h0, hn, blks in halves:
                        a_t = work.tile([128, 1792], BF16, tag="a_t",
                                        name="a_t", bufs=2)
                        s_t = work.tile([128, 1792], BF16, tag="s_t",
                                        name="s_t", bufs=2)
                        border = blks if g == 0 else blks[::-1]
                        for blk in border:
                            lo = (blk - blks[0]) * NT
                            hp = psum.tile([128, NT], F32,
                                           tag=f"P{2 + blk % 2}", name="hp")
                            gp = psum.tile([128, NT], F32,
                                           tag=f"P{4 + blk % 2}", name="gp")
                            for s2 in (0, 1):
                                t0 = blk * NT + s2 * SUB
                                rhs = hc[:, 0:2, t0:t0 + SUB]
                                nc.tensor.matmul(
                                    hp[:, s2 * SUB:(s2 + 1) * SUB],
                                    whg_t[g][:, 0:2, j * 128:(j + 1) * 128],
                                    rhs, start=(s2 == 0), stop=False,
                                    perf_mode=DRM, skip_group_check=True)
                                nc.tensor.matmul(
                                    gp[:, s2 * SUB:(s2 + 1) * SUB],
                                    whg_t[g][:, 0:2,
                                             DI + j * 128:DI + (j + 1) * 128],
                                    rhs, start=(s2 == 0), stop=False,
                                    perf_mode=DRM, skip_group_check=True)
                            sl448 = slice(blk * NT, (blk + 1) * NT)
                            nc.tensor.matmul(
                                hp[:], whg_t[g][:, 2, j * 128:(j + 1) * 128],
                                hc[:, 2, sl448], start=False, stop=True,
                                skip_group_check=True)
                            nc.tensor.matmul(
                                gp[:],
                                whg_t[g][:, 2, DI + j * 128:DI + (j + 1) * 128],
                                hc[:, 2, sl448], start=False, stop=True,
                                skip_group_check=True)
                            nc.scalar.activation(s_t[:, lo:lo + NT], hp[:],
                                                 AF.Sigmoid)
                            nc.scalar.activation(a_t[:, lo:lo + NT], gp[:],
                                                 AF.Sigmoid)
                            # g = max(h + 0.5, sigmoid(h)), in place on s
                            nc.vector.scalar_tensor_tensor(
                                s_t[:, lo:lo + NT], hp[:], 0.5,
                                s_t[:, lo:lo + NT], ALU.add, ALU.max)
                        # b' = (a-1)*g = -b; wout is negated host-side
                        t_t = work.tile([128, 1792], BF16, tag="t_t",
                                        name="t_t", bufs=2)
                        nc.gpsimd.tensor_tensor(t_t[:, 0:hn], a_t[:, 0:hn],
                                                s_t[:, 0:hn], ALU.mult)
                        nc.gpsimd.tensor_tensor(t_t[:, 0:hn], t_t[:, 0:hn],
                                                s_t[:, 0:hn], ALU.subtract)

                        def make_scan(g=g, j=j, h0=h0, hn=hn, a_t=a_t,
                                      t_t=t_t, first=(h0 == 0) == (g == 0)):
                            eng = nc.vector
                            def emit():
                                if g == 0:
                                    init = (0.0 if h0 == 0
                                            else hs_t[0][:, j, 1791:1792])
                                    eng.tensor_tensor_scan(
                                        hs_t[0][:, j, h0:h0 + hn],
                                        a_t[:, 0:hn], t_t[:, 0:hn], init,
                                        ALU.mult, ALU.add)
                                else:
                                    rv = slice(None, None, -1)
                                    init = (0.0 if h0 == 1792
                                            else hs_t[1][:, j, 1792:1793])
                                    eng.tensor_tensor_scan(
                                        hs_t[1][:, j, h0:h0 + hn][:, rv],
                                        a_t[:, 0:hn][:, rv],
                                        t_t[:, 0:hn][:, rv], init,
                                        ALU.mult, ALU.add)
                            return emit

                        if (h0 == 0) == (g == 0):
                            flush_scan()          # flush prev (j,g) pair
                            pending_scan[0] = [make_scan()]
                        else:
                            pending_scan[0].append(make_scan())
            flush_scan()

            # out-projection (bf16) + residual accumulate, in place into x_t
            for g in range(2):
                border = (0, 1, 2, 3, 4, 5, 6) if g == 0 else \
                         (4, 5, 6, 0, 1, 2, 3)
                for blk in border:
                    sl = slice(blk * NT, (blk + 1) * NT)
                    for dc in range(DC):
                        y_ps = psum.tile([128, NT], F32,
                                         tag=f"P{6 + (blk * DC + dc) % 2}",
                                         name="y_ps")
                        for j in range(6):
                            nc.tensor.matmul(
                                y_ps[:],
                                wout_t[g][:, j, dc * 128:(dc + 1) * 128],
                                hs_t[g][:, j, sl],
                                start=(j == 0), stop=(j == 5))
                        nc.vector.tensor_tensor(x_t[dc][:, sl], y_ps[:],
                                                x_t[dc][:, sl], ALU.add)

            # ============================================= norm2 -> yh
            st2 = norm_stats(lambda c, sl: x_t[c][:, sl])

            def yh_pl(c):
                return xh[:, 1 + c * LP: 1 + c * LP + L]

            def store_yh(c, blk, apc, ib):
                sl = slice(blk * NT, (blk + 1) * NT)
                nc.vector.tensor_tensor(yh_pl(c)[:, sl], apc[:], ib[:],
                                        ALU.mult)

            norm_apply(st2, lambda c, sl: x_t[c][:, sl], store_yh)

            def yh_rhs(kk, t0, n):
                return bass.AP(tensor=xf.tensor,
                               offset=xf.offset + 1 + kk * LP + t0,
                               ap=[list(xf.ap[0]), [LP, 2], [1, n]])

            # ============================================= MLP (fp8 DR)
            for blk in range(NBLK):
                sl = slice(blk * NT, (blk + 1) * NT)
                o_ps = [psum.tile([128, NT], F32, tag=f"P{5 + dc}",
                                  name=f"o{dc}") for dc in range(DC)]
                for w in range(3):
                    qt = work.tile([128, 4, NT], FP8, tag="qt", name="qt",
                                   bufs=2)
                    for mi in range(4):
                        mc = 4 * w + mi
                        q_ps = psum.tile([128, NT], F32, tag=f"P{mi}",
                                         name="q_ps")
                        for s2 in (0, 1):
                            nc.tensor.matmul(
                                q_ps[:, s2 * SUB:(s2 + 1) * SUB],
                                p1w[:, 0:2, mc * 128:(mc + 1) * 128],
                                yh_rhs(0, blk * NT + s2 * SUB, SUB),
                                start=(s2 == 0), stop=False,
                                perf_mode=DRM, skip_group_check=True)
                        nc.tensor.matmul(
                            q_ps[:], p1w[:, 2, mc * 128:(mc + 1) * 128],
                            yh_pl(2)[:, sl], start=False, stop=True,
                            skip_group_check=True)
                        nc.scalar.activation(qt[:, mi, :], q_ps[:], AF.Gelu,
                                             scale=1.0 / WS,
                                             bias=p1b_t[:, mc:mc + 1])
                    for dc in range(DC):
                        for mp in (0, 2):
                            for s2 in (0, 1):
                                nc.tensor.matmul(
                                    o_ps[dc][:, s2 * SUB:(s2 + 1) * SUB],
                                    p2w[:, 4 * w + mp:4 * w + mp + 2,
                                        dc * 128:(dc + 1) * 128],
                                    qt[:, mp:mp + 2, s2 * SUB:(s2 + 1) * SUB],
                                    start=(w == 0 and mp == 0 and s2 == 0),
                                    stop=False,
                                    perf_mode=DRM, skip_group_check=True)
                oo = []
                for dc in range(DC):
                    # + p2b (pre-scaled by WS) via K=1 matmul
                    nc.tensor.matmul(o_ps[dc][:],
                                     p2bT_t[:, dc * 128:(dc + 1) * 128],
                                     onesN[:], start=False, stop=True,
                                     skip_group_check=True)
                    ot = work.tile([128, NT], F32, tag="oo", name="oo", bufs=2)
                    nc.vector.scalar_tensor_tensor(ot[:], o_ps[dc][:],
                                                   1.0 / WS, x_t[dc][:, sl],
                                                   ALU.mult, ALU.add)
                    oo.append(ot)
                for dc in range(DC):
                    nc.sync.dma_start(out_d[b, dc * 128:(dc + 1) * 128, sl],
                                      oo[dc][:])

    return nc


# ------------------------------------------------------------------ host side
_NC_CACHE = {}


def _get_nc():
    if "nc" not in _NC_CACHE:
        nc = build_kernel()
        _fix_multiwaits(nc)
        _NC_CACHE["nc"] = nc
    return _NC_CACHE["nc"]


def _q8(a, scale=1.0):
    return np.clip(np.asarray(a, np.float32) * scale, -240.0,
                   240.0).astype(ml_dtypes.float8_e4m3)


def _prep_weights(inp):
    f = np.float32
    norm_w = np.asarray(inp["norm_w"], f)
    norm_b = np.asarray(inp["norm_b"], f)
    dw_w = np.asarray(inp["dw_w"], f)[:, 0]              # [D,3,3]
    dw_wf = dw_w * norm_w[:, None, None]
    dw_bf = np.asarray(inp["dw_b"], f) + norm_b * dw_w.sum(axis=(1, 2))
    p1_w = np.asarray(inp["p1_w"], f)
    p1f = p1_w * np.asarray(inp["norm2_w"], f)[:, None]
    p1bf = np.asarray(inp["p1_b"], f) + np.asarray(inp["norm2_b"], f) @ p1_w

    # GRU in-proj: [D, 2DI] -> [128, 3, 1536], gate half negated
    whg8 = np.zeros((2, 128, 3, 1536), ml_dtypes.float8_e4m3)
    for g, key in enumerate(["gru1_whg", "gru2_whg"]):
        w = np.asarray(inp[key], f).copy()               # [384, 1536]
        w[:, DI:] = -w[:, DI:]
        for k in range(3):
            whg8[g, :, k, :] = _q8(w[k * 128:(k + 1) * 128, :])
    # GRU out-proj bf16, NEGATED (the scan produces -h): [DI, D] -> [128,6,D]
    wout16 = np.zeros((2, 128, 6, D), ml_dtypes.bfloat16)
    for g, key in enumerate(["gru1_wout", "gru2_wout"]):
        w = -np.asarray(inp[key], f)
        for k in range(6):
            wout16[g, :, k, :] = w[k * 128:(k + 1) * 128, :].astype(
                ml_dtypes.bfloat16)
    # MLP
    p18 = np.zeros((128, 3, MLPD), ml_dtypes.float8_e4m3)
    for k in range(3):
        p18[:, k, :] = _q8(p1f[k * 128:(k + 1) * 128, :], WS)
    p28 = np.zeros((128, 12, D), ml_dtypes.float8_e4m3)
    p2 = np.asarray(inp["p2_w"], f)
    for k in range(12):
        p28[:, k, :] = _q8(p2[k * 128:(k + 1) * 128, :], WS)
    # conv diag blocks, slot order [4, 0,2, 3,5, 6,8, 1,7]
    slots = [4, 0, 2, 3, 5, 6, 8, 1, 7]
    cw8 = np.zeros((DC, 128, 9, 128), ml_dtypes.float8_e4m3)
    ar = np.arange(128)
    for c in range(DC):
        for si, tap in enumerate(slots):
            dr, dcc = tap // 3, tap % 3
            cw8[c, ar, si, ar] = _q8(
                dw_wf[c * 128:(c + 1) * 128, dr, dcc], CONVS)

    return dict(
        whg8=whg8, wout16=wout16, p18=p18, p28=p28, convw8=cw8,
        dwb=np.ascontiguousarray(dw_bf.reshape(DC, 128).T, f),
        p1b=np.ascontiguousarray(p1bf.reshape(12, 128).T, f),
        p2bT=np.ascontiguousarray(
            (np.asarray(inp["p2_b"], f) * WS).reshape(1, D)).astype(
                ml_dtypes.bfloat16),
        ident=np.eye(128, dtype=f).astype(ml_dtypes.bfloat16),
    )


def kernel(**inputs):
    x = np.asarray(inputs["x"], np.float32)              # [16, L, D]
    w = _prep_weights(inputs)
    nc = _get_nc()

    in_maps = []
    for core in range(NCORES):
        xb = x[core * B:(core + 1) * B]                  # [B, L, D]
        m = dict(w)
        m["xT"] = np.ascontiguousarray(xb.transpose(0, 2, 1))
        in_maps.append(m)

    res = run_bass_kernel_spmd(nc, in_maps, core_ids=list(range(NCORES)))
    outs = []
    for core in range(NCORES):
        oT = res.results[core]["outT"]                   # [B, D, L]
        outs.append(oT.transpose(0, 2, 1))
    return np.ascontiguousarray(np.concatenate(outs, axis=0), np.float32)


# revision 11
# speedup vs baseline: 1.1625x; 1.1625x over previous
"""Trainium2 Bass kernel for nn_Block2DGRU (norm->dwconv3x3->bi-minGRU->norm->MLP).

fp8e4m3 DoubleRow matmuls for the heavy GEMMs (GRU in-proj, MLP p1/p2, conv),
bf16 for the GRU out-projection (scan-output fp8 quantization dominates the
error budget), f32r for layernorm stats/broadcast matmuls.

Layout: [feature_on_partitions, time_on_free], 8 NeuronCores SPMD, 2 batch
items per core.  Per batch item:
  - x_t:   3x[128, L] f32 input chunks; overwritten in place by the GRU
           residual accumulation (y = gru1 + gru2 + x).
  - xh:    fp8 normalized input, 4 K-planes (K padded 384->512 for DoubleRow),
           row pitch 57 (zero pad column) so conv column shifts are plain +-1
           element offsets; 1-element zero guard in front.
  - hc:    [128, 4, L] fp8 conv output (contiguous, plane 3 zero).
  - hs:    [128, 6, L] bf16 scan outputs per GRU.
  - yh:    fp8 norm2 output stored in the xh slot (contiguous plane slices).
Gate-half GRU weights are pre-negated so sigmoid() yields a = 1-z directly;
b = z*g = g - a*g is built on GpSimd.
"""
import numpy as np
import ml_dtypes

import concourse.bass as bass
import concourse.tile as tile
import concourse.mybir as mybir
from concourse.bass_utils import run_bass_kernel_spmd

F32 = mybir.dt.float32
F32R = mybir.dt.float32r
BF16 = mybir.dt.bfloat16
FP8 = mybir.dt.float8e4
AF = mybir.ActivationFunctionType
ALU = mybir.AluOpType
DRM = mybir.MatmulPerfMode.DoubleRow

NB = 56
NBP = 57                     # padded row pitch
L = NB * NB                  # 3136
LP = NBP * NB                # 3192
D = 384
DC = 3
DI = 768
MLPD = 1536
B = 2
NCORES = 8
NT = 448                     # time block = 8 image rows
NBLK = 7
SUB = 224                    # DoubleRow moving sub-block
NTP = 8 * NBP                # 456: padded conv psum block
HALVES = [(0, 1792, (0, 1, 2, 3)), (1792, 1344, (4, 5, 6))]
STATS_GROUPS = [(0, 1, 2), (3, 4, 5), (6,)]
EPS = 1e-5
WS = 32.0                    # fp8 scale for p1/p2 weights (+WS*p2b fold)
CONVS = 16.0                 # conv weight scale


def _fix_multiwaits(nc):
    """The walrus accepts at most ONE sync wait per instruction; hoist
    extras into wait-only NoOps on the same engine (streams are in-order)."""
    n = 0
    cnt = [0]
    for f in nc.m.functions:
        for bb in f.blocks:
            out = []
            for inst in bb.instructions:
                si = inst.sync_info
                if si is not None and si.on_wait is not None and len(si.on_wait) > 1:
                    waits = list(si.on_wait)
                    for w in waits[:-1]:
                        cnt[0] += 1
                        nop = mybir.InstNoOp(
                            name=f"I-waitfix-{cnt[0]}",
                            sync_info=mybir.SyncInfo(on_wait=[w], on_update=[]),
                        )
                        nop.engine = inst.engine
                        out.append(nop)
                    inst.sync_info = mybir.SyncInfo(
                        on_wait=[waits[-1]], on_update=list(si.on_update or [])
                    )
                    n += 1
                out.append(inst)
            bb.instructions = out
    return n


def build_kernel():
    nc = bass.Bass("TRN2", target_bir_lowering=False, debug=False,
                   num_devices=NCORES)

    xT_d = nc.dram_tensor("xT", [B, D, L], F32, kind="ExternalInput").ap()
    whg_d = nc.dram_tensor("whg8", [2, 128, 3, 1536], FP8,
                           kind="ExternalInput").ap()
    wout_d = nc.dram_tensor("wout16", [2, 128, 6, D], BF16,
                            kind="ExternalInput").ap()
    p1_d = nc.dram_tensor("p18", [128, 3, MLPD], FP8, kind="ExternalInput").ap()
    p2_d = nc.dram_tensor("p28", [128, 12, D], FP8, kind="ExternalInput").ap()
    cw_d = nc.dram_tensor("convw8", [DC, 128, 9, 128], FP8,
                          kind="ExternalInput").ap()
    dwb_d = nc.dram_tensor("dwb", [128, DC], F32, kind="ExternalInput").ap()
    p1b_d = nc.dram_tensor("p1b", [128, 12], F32, kind="ExternalInput").ap()
    p2bT_d = nc.dram_tensor("p2bT", [1, D], BF16, kind="ExternalInput").ap()
    ident_d = nc.dram_tensor("ident", [128, 128], BF16,
                             kind="ExternalInput").ap()
    out_d = nc.dram_tensor("outT", [B, D, L], F32, kind="ExternalOutput").ap()

    from contextlib import ExitStack
    with tile.TileContext(nc) as tc, ExitStack() as ctx:
        big = ctx.enter_context(tc.tile_pool(name="big", bufs=1))
        wpool = ctx.enter_context(tc.tile_pool(name="wpool", bufs=1))
        work = ctx.enter_context(tc.tile_pool(name="work", bufs=2))
        psum = ctx.enter_context(tc.tile_pool(name="psum", bufs=1, space="PSUM"))

        # ---------------- persistent weights / constants (loaded once)
        whg_t = [wpool.tile([128, 3, 1536], FP8, tag=f"whg{g}", name=f"whg{g}")
                 for g in range(2)]
        wout_t = [wpool.tile([128, 6, D], BF16, tag=f"wo{g}", name=f"wo{g}")
                  for g in range(2)]
        for g in range(2):
            nc.sync.dma_start(whg_t[g][:], whg_d[g])
            nc.sync.dma_start(wout_t[g][:], wout_d[g])
        p1w = wpool.tile([128, 3, MLPD], FP8, tag="p1w", name="p1w")
        nc.sync.dma_start(p1w[:], p1_d)
        p2w = wpool.tile([128, 12, D], FP8, tag="p2w", name="p2w")
        nc.sync.dma_start(p2w[:], p2_d)
        cw_t = [wpool.tile([128, 9, 128], FP8, tag=f"cw{c}", name=f"cw{c}")
                for c in range(DC)]
        for c in range(DC):
            nc.sync.dma_start(cw_t[c][:], cw_d[c])
        dwb_t = wpool.tile([128, DC], F32, tag="dwb", name="dwb")
        nc.sync.dma_start(dwb_t[:], dwb_d)
        p1b_t = wpool.tile([128, 12], F32, tag="p1b", name="p1b")
        nc.sync.dma_start(p1b_t[:], p1b_d)
        p2bT_t = wpool.tile([1, D], BF16, tag="p2bT", name="p2bT")
        nc.sync.dma_start(p2bT_t[:], p2bT_d)
        ident_t = wpool.tile([128, 128], BF16, tag="ident", name="ident")
        nc.sync.dma_start(ident_t[:], ident_d)

        zsel16 = wpool.tile([128, 129], BF16, tag="zsel16", name="zsel16")
        nc.vector.memset(zsel16[:], 0.0)
        nc.vector.memset(zsel16[:, 64:65], 1.0)
        ones1 = wpool.tile([65, 128], BF16, tag="ones1", name="ones1")
        nc.vector.memset(ones1[:], 1.0)
        onesN = wpool.tile([1, NT], BF16, tag="onesN", name="onesN")
        nc.vector.memset(onesN[:], 1.0)
        eps_t = wpool.tile([128, 1], F32, tag="eps", name="eps")
        nc.vector.memset(eps_t[:], EPS)

        def r32(ap):
            return ap.bitcast(F32R)

        for b in range(B):
            # ---------------- batch-persistent tiles
            x_t = [big.tile([128, L], BF16, tag=f"x{c}", name=f"x{c}",
                            bufs=2) for c in range(DC)]
            for c in range(DC):
                nc.gpsimd.dma_start(x_t[c][:],
                                    xT_d[b, c * 128:(c + 1) * 128, :])
            # padded fp8 activations: 1 guard element + 4 planes of LP
            xh = big.tile([128, 1 + 3 * LP], FP8, tag="act8", name="act8")
            xf = xh[:]
            nc.gpsimd.memset(xh[:, 0:1], 0.0)              # guard
            # zero the pad column (index 56 of each 57-row) of planes 0..2
            pads = bass.AP(tensor=xf.tensor, offset=xf.offset + 1 + NB,
                           ap=[list(xf.ap[0]), [LP, 3], [NBP, NB]])
            nc.gpsimd.memset(pads, 0.0)

            def xh_pl(c):
                return xh[:, 1 + c * LP: 1 + (c + 1) * LP]

            hc = big.tile([128, 3, L], FP8, tag="hc", name="hc")

            # ============================================= layernorm helper
            def norm_stats(src):
                """src(c, sl) -> f32 AP.  Returns [(numu, inv, blks), ...]
                with per-block stats rows at partitions {0,32,64,96}."""
                res = []
                for grp, blks in enumerate(STATS_GROUPS):
                    sA = psum.tile([65, NT], F32, tag="P0", name="sA")
                    sB = psum.tile([65, NT], F32, tag="P1", name="sB")
                    nlast = len(blks) - 1
                    for i, blk in enumerate(blks):
                        sl = slice(blk * NT, (blk + 1) * NT)
                        sq = work.tile([128, NT], BF16, tag="sq", name="sq",
                                       bufs=1)
                        for c in range(DC):
                            nc.scalar.activation(sq[:], src(c, sl), AF.Square)
                            nc.tensor.matmul(
                                sA[:], zsel16[:, 64 - 32 * i:129 - 32 * i],
                                src(c, sl), start=(i == 0 and c == 0),
                                stop=(i == nlast and c == DC - 1),
                                skip_group_check=True)
                            nc.tensor.matmul(
                                sB[:], zsel16[:, 64 - 32 * i:129 - 32 * i],
                                sq[:], start=(i == 0 and c == 0),
                                stop=(i == nlast and c == DC - 1),
                                skip_group_check=True)
                    numu = work.tile([65, NT], BF16, tag=f"numu{grp}",
                                     name=f"numu{grp}", bufs=1)
                    nc.scalar.activation(numu[:], sA[:], AF.Copy,
                                         scale=-1.0 / D)
                    scr = work.tile([65, NT], F32, tag="nscr", name="nscr",
                                    bufs=1)
                    nc.scalar.activation(scr[:], sA[:], AF.Square,
                                         scale=1.0 / D)
                    nc.vector.scalar_tensor_tensor(scr[:], sB[:], 1.0 / D,
                                                   scr[:], ALU.mult,
                                                   ALU.subtract)
                    nc.scalar.activation(scr[:], scr[:], AF.Sqrt,
                                         bias=eps_t[0:65, :])
                    scr2 = work.tile([65, NT], F32, tag="nscr2",
                                     name="nscr2", bufs=1)
                    nc.vector.reciprocal(scr2[:], scr[:])
                    inv = work.tile([65, NT], BF16, tag=f"inv{grp}",
                                    name=f"inv{grp}", bufs=1)
                    nc.vector.tensor_copy(inv[:], scr2[:])
                    res.append((numu, inv, blks))
                return res

            def norm_apply(stats, src, store):
                for numu, inv, blks in stats:
                    for i, blk in enumerate(blks):
                        sl = slice(blk * NT, (blk + 1) * NT)
                        ib = psum.tile([128, NT], F32, tag="P4", name="ib")
                        nc.tensor.matmul(ib[:],
                                         ones1[32 * i:32 * i + 1, :],
                                         inv[32 * i:32 * i + 1, :],
                                         start=True, stop=True)
                        ibs = work.tile([128, NT], BF16, tag="ibs",
                                        name="ibs", bufs=2)
                        nc.scalar.activation(ibs[:], ib[:], AF.Copy)
                        for c in range(DC):
                            apc = psum.tile([128, NT], F32,
                                            tag=f"P{2 + c % 2}", name="apc")
                            nc.tensor.matmul(apc[:],
                                             ones1[32 * i:32 * i + 1, :],
                                             numu[32 * i:32 * i + 1, :],
                                             start=True, stop=False,
                                             skip_group_check=True)
                            nc.tensor.matmul(apc[:], ident_t[:],
                                             src(c, sl),
                                             start=False, stop=True,
                                             skip_group_check=True)
                            store(c, blk, apc, ibs)

            # ------------------------------------------------ norm1 -> xh
            st1 = norm_stats(lambda c, sl: x_t[c][:, sl])

            def store_xh(c, blk, apc, ib):
                base = xh_pl(c)
                dst = bass.AP(tensor=base.tensor,
                              offset=base.offset + blk * 8 * NBP,
                              ap=[list(base.ap[0]), [NBP, 8], [1, NB]])
                src3 = apc[:].rearrange("p (rr cc) -> p rr cc", cc=NB)
                ib3 = ib[:].rearrange("p (rr cc) -> p rr cc", cc=NB)
                nc.vector.tensor_tensor(dst, src3, ib3, ALU.mult)

            norm_apply(st1, lambda c, sl: x_t[c][:, sl], store_xh)

            # ============================================= depthwise conv 3x3
            # cw slot order: [t4, t0,t2, t3,t5, t6,t8, t1,t7]
            for c in range(DC):
                base = xh_pl(c)
                pstride = list(base.ap[0])

                def mv(off, d1, n):
                    return bass.AP(tensor=base.tensor,
                                   offset=base.offset + off,
                                   ap=[pstride, [d1, 2], [1, n]])

                for slab in range(NBLK):
                    cp = psum.tile([128, NTP], F32,
                                   tag=("P6" if slab % 2 == 0 else "P7"),
                                   name="cp")
                    r0 = 8 * slab
                    # solo center tap (0,0): full padded range, starts group
                    for o0, o1 in ((0, 228), (228, NTP)):
                        nc.tensor.matmul(cp[:, o0:o1], cw_t[c][:, 0, :],
                                         base[:, r0 * NBP + o0:r0 * NBP + o1],
                                         start=(o0 == 0), stop=False,
                                         skip_group_check=True)
                    # column pairs (dr,-1)&(dr,+1): dim1 stride 2
                    for dr, slot in ((-1, 1), (0, 3), (1, 5)):
                        lo = max(0, -(r0 + dr))
                        hi = max(0, (r0 + 8 + dr) - NB)
                        orow0, orow1 = lo, 8 - hi
                        ncols = (orow1 - orow0) * NBP - 1
                        src0 = (r0 + orow0 + dr) * NBP - 1
                        mid = ncols // 2
                        for c0, c1 in ((0, mid), (mid, ncols)):
                            nc.tensor.matmul(
                                cp[:, orow0 * NBP + c0:orow0 * NBP + c1],
                                cw_t[c][:, slot:slot + 2, :],
                                mv(src0 + c0, 2, c1 - c0),
                                start=False, stop=False, perf_mode=DRM,
                                skip_group_check=True)
                    # row pair (-1,0)&(+1,0): dim1 stride 2*NBP
                    lo = max(0, 1 - r0)
                    hi = max(0, (r0 + 8) - (NB - 1))
                    orow0, orow1 = lo, 8 - hi
                    ncols = (orow1 - orow0) * NBP - 1
                    src0 = (r0 + orow0 - 1) * NBP
                    mid = ncols // 2
                    interior = slab not in (0, NBLK - 1)
                    for si, (c0, c1) in enumerate(((0, mid), (mid, ncols))):
                        nc.tensor.matmul(
                            cp[:, orow0 * NBP + c0:orow0 * NBP + c1],
                            cw_t[c][:, 7:9, :], mv(src0 + c0, 2 * NBP, c1 - c0),
                            start=False, stop=(interior and si == 1),
                            perf_mode=DRM, skip_group_check=True)
                    # slab edge rows 0 / 55 get only one of the (+-1,0) taps
                    if slab == 0:
                        nc.tensor.matmul(cp[:, 0:NB], cw_t[c][:, 8, :],
                                         base[:, NBP:NBP + NB],
                                         start=False, stop=True,
                                         skip_group_check=True)
                    if slab == NBLK - 1:
                        nc.tensor.matmul(cp[:, 7 * NBP:7 * NBP + NB],
                                         cw_t[c][:, 7, :],
                                         base[:, 54 * NBP:54 * NBP + NB],
                                         start=False, stop=True,
                                         skip_group_check=True)
                    # readout: drop pad columns, descale, +bias -> fp8
                    cin = cp[:].rearrange("p (rr cc) -> p rr cc",
                                          cc=NBP)[:, :, 0:NB]
                    hout = hc[:, c, slab * NT:(slab + 1) * NT].rearrange(
                        "p (rr cc) -> p rr cc", cc=NB)
                    nc.scalar.activation(hout, cin, AF.Identity,
                                         scale=1.0 / CONVS,
                                         bias=dwb_t[:, c:c + 1])

            # ============================================= bidirectional GRU
            hs_t = [big.tile([128, 6, L], BF16, tag=f"hs{g}", name=f"hs{g}")
                    for g in range(2)]
            pending_scan = [None]

            def flush_scan():
                if pending_scan[0] is not None:
                    for emit in pending_scan[0]:
                        emit()
                    pending_scan[0] = None

            for j in range(6):
                for g in range(2):
                    halves = HALVES if g == 0 else HALVES[::-1]
                    for h0, hn, blks in halves:
                        a_t = work.tile([128, 1792], BF16, tag="a_t",
                                        name="a_t", bufs=2)
                        s_t = work.tile([128, 1792], BF16, tag="s_t",
                                        name="s_t", bufs=2)
                        border = blks if g == 0 else blks[::-1]
                        for blk in border:
                            lo = (blk - blks[0]) * NT
                            hp = psum.tile([128, NT], F32,
                                           tag=f"P{2 + blk % 2}", name="hp")
                            gp = psum.tile([128, NT], F32,
                                           tag=f"P{4 + blk % 2}", name="gp")
                            for s2 in (0, 1):
                                t0 = blk * NT + s2 * SUB
                                rhs = hc[:, 0:2, t0:t0 + SUB]
                                nc.tensor.matmul(
                                    hp[:, s2 * SUB:(s2 + 1) * SUB],
                                    whg_t[g][:, 0:2, j * 128:(j + 1) * 128],
                                    rhs, start=(s2 == 0), stop=False,
                                    perf_mode=DRM, skip_group_check=True)
                                nc.tensor.matmul(
                                    gp[:, s2 * SUB:(s2 + 1) * SUB],
                                    whg_t[g][:, 0:2,
                                             DI + j * 128:DI + (j + 1) * 128],
                                    rhs, start=(s2 == 0), stop=False,
                                    perf_mode=DRM, skip_group_check=True)
                            sl448 = slice(blk * NT, (blk + 1) * NT)
                            nc.tensor.matmul(
                                hp[:], whg_t[g][:, 2, j * 128:(j + 1) * 128],
                                hc[:, 2, sl448], start=False, stop=True,
                                skip_group_check=True)
                            nc.tensor.matmul(
                                gp[:],
                                whg_t[g][:, 2, DI + j * 128:DI + (j + 1) * 128],
                                hc[:, 2, sl448], start=False, stop=True,
                                skip_group_check=True)
                            nc.scalar.activation(s_t[:, lo:lo + NT], hp[:],
                                                 AF.Sigmoid)
                            nc.scalar.activation(a_t[:, lo:lo + NT], gp[:],
                                                 AF.Sigmoid)
                            # g = max(h + 0.5, sigmoid(h)), in place on s
                            nc.vector.scalar_tensor_tensor(
                                s_t[:, lo:lo + NT], hp[:], 0.5,
                                s_t[:, lo:lo + NT], ALU.add, ALU.max)
                        # b' = (a-1)*g = -b; wout is negated host-side
                        t_t = work.tile([128, 1792], BF16, tag="t_t",
                                        name="t_t", bufs=2)
                        nc.vector.scalar_tensor_tensor(
                            t_t[:, 0:hn], a_t[:, 0:hn], 1.0, s_t[:, 0:hn],
                            ALU.subtract, ALU.mult)

                        def make_scan(g=g, j=j, h0=h0, hn=hn, a_t=a_t,
                                      t_t=t_t, first=(h0 == 0) == (g == 0)):
                            eng = nc.vector
                            def emit():
                                if g == 0:
                                    init = (0.0 if h0 == 0
                                            else hs_t[0][:, j, 1791:1792])
                                    eng.tensor_tensor_scan(
                                        hs_t[0][:, j, h0:h0 + hn],
                                        a_t[:, 0:hn], t_t[:, 0:hn], init,
                                        ALU.mult, ALU.add)
                                else:
                                    rv = slice(None, None, -1)
                                    init = (0.0 if h0 == 1792
                                            else hs_t[1][:, j, 1792:1793])
                                    eng.tensor_tensor_scan(
                                        hs_t[1][:, j, h0:h0 + hn][:, rv],
                                        a_t[:, 0:hn][:, rv],
                                        t_t[:, 0:hn][:, rv], init,
                                        ALU.mult, ALU.add)
                            return emit

                        if (h0 == 0) == (g == 0):
                            flush_scan()          # flush prev (j,g) pair
                            pending_scan[0] = [make_scan()]
                        else:
                            pending_scan[0].append(make_scan())
            flush_scan()

            # out-projection (bf16) + residual accumulate, in place into x_t
            for g in range(2):
                border = (0, 1, 2, 3, 4, 5, 6) if g == 0 else \
                         (4, 5, 6, 0, 1, 2, 3)
                for blk in border:
                    sl = slice(blk * NT, (blk + 1) * NT)
                    for dc in range(DC):
                        y_ps = psum.tile([128, NT], F32,
                                         tag=f"P{6 + (blk * DC + dc) % 2}",
                                         name="y_ps")
                        for j in range(6):
                            nc.tensor.matmul(
                                y_ps[:],
                                wout_t[g][:, j, dc * 128:(dc + 1) * 128],
                                hs_t[g][:, j, sl],
                                start=(j == 0), stop=(j == 5))
                        nc.vector.tensor_tensor(x_t[dc][:, sl], y_ps[:],
                                                x_t[dc][:, sl], ALU.add)

            # ============================================= norm2 -> yh
            st2 = norm_stats(lambda c, sl: x_t[c][:, sl])

            def yh_pl(c):
                return xh[:, 1 + c * LP: 1 + c * LP + L]

            def store_yh(c, blk, apc, ib):
                sl = slice(blk * NT, (blk + 1) * NT)
                nc.vector.tensor_tensor(yh_pl(c)[:, sl], apc[:], ib[:],
                                        ALU.mult)

            norm_apply(st2, lambda c, sl: x_t[c][:, sl], store_yh)

            def yh_rhs(kk, t0, n):
                return bass.AP(tensor=xf.tensor,
                               offset=xf.offset + 1 + kk * LP + t0,
                               ap=[list(xf.ap[0]), [LP, 2], [1, n]])

            # ============================================= MLP (fp8 DR)
            for blk in range(NBLK):
                sl = slice(blk * NT, (blk + 1) * NT)
                o_ps = [psum.tile([128, NT], F32, tag=f"P{5 + dc}",
                                  name=f"o{dc}") for dc in range(DC)]
                for w in range(3):
                    qt = work.tile([128, 4, NT], FP8, tag="qt", name="qt",
                                   bufs=2)
                    for mi in range(4):
                        mc = 4 * w + mi
                        q_ps = psum.tile([128, NT], F32, tag=f"P{mi}",
                                         name="q_ps")
                        for s2 in (0, 1):
                            nc.tensor.matmul(
                                q_ps[:, s2 * SUB:(s2 + 1) * SUB],
                                p1w[:, 0:2, mc * 128:(mc + 1) * 128],
                                yh_rhs(0, blk * NT + s2 * SUB, SUB),
                                start=(s2 == 0), stop=False,
                                perf_mode=DRM, skip_group_check=True)
                        nc.tensor.matmul(
                            q_ps[:], p1w[:, 2, mc * 128:(mc + 1) * 128],
                            yh_pl(2)[:, sl], start=False, stop=True,
                            skip_group_check=True)
                        nc.scalar.activation(qt[:, mi, :], q_ps[:], AF.Gelu,
                                             scale=1.0 / WS,
                                             bias=p1b_t[:, mc:mc + 1])
                    for dc in range(DC):
                        for mp in (0, 2):
                            for s2 in (0, 1):
                                nc.tensor.matmul(
                                    o_ps[dc][:, s2 * SUB:(s2 + 1) * SUB],
                                    p2w[:, 4 * w + mp:4 * w + mp + 2,
                                        dc * 128:(dc + 1) * 128],
                                    qt[:, mp:mp + 2, s2 * SUB:(s2 + 1) * SUB],
                                    start=(w == 0 and mp == 0 and s2 == 0),
                                    stop=False,
                                    perf_mode=DRM, skip_group_check=True)
                oo = []
                for dc in range(DC):
                    # + p2b (pre-scaled by WS) via K=1 matmul
                    nc.tensor.matmul(o_ps[dc][:],
                                     p2bT_t[:, dc * 128:(dc + 1) * 128],
                                     onesN[:], start=False, stop=True,
                                     skip_group_check=True)
                    ot = work.tile([128, NT], F32, tag="oo", name="oo", bufs=2)
                    nc.vector.scalar_tensor_tensor(ot[:], o_ps[dc][:],
                                                   1.0 / WS, x_t[dc][:, sl],
                                                   ALU.mult, ALU.add)
                    oo.append(ot)
                for dc in range(DC):
                    nc.sync.dma_start(out_d[b, dc * 128:(dc + 1) * 128, sl],
                                      oo[dc][:])

    return nc


# ------------------------------------------------------------------ host side
_NC_CACHE = {}


def _get_nc():
    if "nc" not in _NC_CACHE:
        nc = build_kernel()
        _fix_multiwaits(nc)
        _NC_CACHE["nc"] = nc
    return _NC_CACHE["nc"]


def _q8(a, scale=1.0):
    return np.clip(np.asarray(a, np.float32) * scale, -240.0,
                   240.0).astype(ml_dtypes.float8_e4m3)


def _prep_weights(inp):
    f = np.float32
    norm_w = np.asarray(inp["norm_w"], f)
    norm_b = np.asarray(inp["norm_b"], f)
    dw_w = np.asarray(inp["dw_w"], f)[:, 0]              # [D,3,3]
    dw_wf = dw_w * norm_w[:, None, None]
    dw_bf = np.asarray(inp["dw_b"], f) + norm_b * dw_w.sum(axis=(1, 2))
    p1_w = np.asarray(inp["p1_w"], f)
    p1f = p1_w * np.asarray(inp["norm2_w"], f)[:, None]
    p1bf = np.asarray(inp["p1_b"], f) + np.asarray(inp["norm2_b"], f) @ p1_w

    # GRU in-proj: [D, 2DI] -> [128, 3, 1536], gate half negated
    whg8 = np.zeros((2, 128, 3, 1536), ml_dtypes.float8_e4m3)
    for g, key in enumerate(["gru1_whg", "gru2_whg"]):
        w = np.asarray(inp[key], f).copy()               # [384, 1536]
        w[:, DI:] = -w[:, DI:]
        for k in range(3):
            whg8[g, :, k, :] = _q8(w[k * 128:(k + 1) * 128, :])
    # GRU out-proj bf16, NEGATED (the scan produces -h): [DI, D] -> [128,6,D]
    wout16 = np.zeros((2, 128, 6, D), ml_dtypes.bfloat16)
    for g, key in enumerate(["gru1_wout", "gru2_wout"]):
        w = -np.asarray(inp[key], f)
        for k in range(6):
            wout16[g, :, k, :] = w[k * 128:(k + 1) * 128, :].astype(
                ml_dtypes.bfloat16)
    # MLP
    p18 = np.zeros((128, 3, MLPD), ml_dtypes.float8_e4m3)
    for k in range(3):
        p18[:, k, :] = _q8(p1f[k * 128:(k + 1) * 128, :], WS)
    p28 = np.zeros((128, 12, D), ml_dtypes.float8_e4m3)
    p2 = np.asarray(inp["p2_w"], f)
    for k in range(12):
        p28[:, k, :] = _q8(p2[k * 128:(k + 1) * 128, :], WS)
    # conv diag blocks, slot order [4, 0,2, 3,5, 6,8, 1,7]
    slots = [4, 0, 2, 3, 5, 6, 8, 1, 7]
    cw8 = np.zeros((DC, 128, 9, 128), ml_dtypes.float8_e4m3)
    ar = np.arange(128)
    for c in range(DC):
        for si, tap in enumerate(slots):
            dr, dcc = tap // 3, tap % 3
            cw8[c, ar, si, ar] = _q8(
                dw_wf[c * 128:(c + 1) * 128, dr, dcc], CONVS)

    return dict(
        whg8=whg8, wout16=wout16, p18=p18, p28=p28, convw8=cw8,
        dwb=np.ascontiguousarray(dw_bf.reshape(DC, 128).T, f),
        p1b=np.ascontiguousarray(p1bf.reshape(12, 128).T, f),
        p2bT=np.ascontiguousarray(
            (np.asarray(inp["p2_b"], f) * WS).reshape(1, D)).astype(
                ml_dtypes.bfloat16),
        ident=np.eye(128, dtype=f).astype(ml_dtypes.bfloat16),
    )


def kernel(**inputs):
    x = np.asarray(inputs["x"], np.float32)              # [16, L, D]
    w = _prep_weights(inputs)
    nc = _get_nc()

    in_maps = []
    for core in range(NCORES):
        xb = x[core * B:(core + 1) * B]                  # [B, L, D]
        m = dict(w)
        m["xT"] = np.ascontiguousarray(xb.transpose(0, 2, 1))
        in_maps.append(m)

    res = run_bass_kernel_spmd(nc, in_maps, core_ids=list(range(NCORES)))
    outs = []
    for core in range(NCORES):
        oT = res.results[core]["outT"]                   # [B, D, L]
        outs.append(oT.transpose(0, 2, 1))
    return np.ascontiguousarray(np.concatenate(outs, axis=0), np.float32)


# revision 12
# speedup vs baseline: 1.1724x; 1.0085x over previous
"""Trainium2 Bass kernel for nn_Block2DGRU (norm->dwconv3x3->bi-minGRU->norm->MLP).

fp8e4m3 DoubleRow matmuls for the heavy GEMMs (GRU in-proj, MLP p1/p2, conv),
bf16 for the GRU out-projection (scan-output fp8 quantization dominates the
error budget), f32r for layernorm stats/broadcast matmuls.

Layout: [feature_on_partitions, time_on_free], 8 NeuronCores SPMD, 2 batch
items per core.  Per batch item:
  - x_t:   3x[128, L] f32 input chunks; overwritten in place by the GRU
           residual accumulation (y = gru1 + gru2 + x).
  - xh:    fp8 normalized input, 4 K-planes (K padded 384->512 for DoubleRow),
           row pitch 57 (zero pad column) so conv column shifts are plain +-1
           element offsets; 1-element zero guard in front.
  - hc:    [128, 4, L] fp8 conv output (contiguous, plane 3 zero).
  - hs:    [128, 6, L] bf16 scan outputs per GRU.
  - yh:    fp8 norm2 output stored in the xh slot (contiguous plane slices).
Gate-half GRU weights are pre-negated so sigmoid() yields a = 1-z directly;
b = z*g = g - a*g is built on GpSimd.
"""
import numpy as np
import ml_dtypes

import concourse.bass as bass
import concourse.tile as tile
import concourse.mybir as mybir
from concourse.bass_utils import run_bass_kernel_spmd

F32 = mybir.dt.float32
F32R = mybir.dt.float32r
BF16 = mybir.dt.bfloat16
FP8 = mybir.dt.float8e4
AF = mybir.ActivationFunctionType
ALU = mybir.AluOpType
DRM = mybir.MatmulPerfMode.DoubleRow

NB = 56
NBP = 57                     # padded row pitch
L = NB * NB                  # 3136
LP = NBP * NB                # 3192
D = 384
DC = 3
DI = 768
MLPD = 1536
B = 2
NCORES = 8
NT = 448                     # time block = 8 image rows
NBLK = 7
SUB = 224                    # DoubleRow moving sub-block
NTP = 8 * NBP                # 456: padded conv psum block
HALVES = [(0, 1792, (0, 1, 2, 3)), (1792, 1344, (4, 5, 6))]
STATS_GROUPS = [(0, 1, 2), (3, 4, 5), (6,)]
EPS = 1e-5
WS = 32.0                    # fp8 scale for p1/p2 weights (+WS*p2b fold)
CONVS = 16.0                 # conv weight scale


def _fix_multiwaits(nc):
    """The walrus accepts at most ONE sync wait per instruction; hoist
    extras into wait-only NoOps on the same engine (streams are in-order)."""
    n = 0
    cnt = [0]
    for f in nc.m.functions:
        for bb in f.blocks:
            out = []
            for inst in bb.instructions:
                si = inst.sync_info
                if si is not None and si.on_wait is not None and len(si.on_wait) > 1:
                    waits = list(si.on_wait)
                    for w in waits[:-1]:
                        cnt[0] += 1
                        nop = mybir.InstNoOp(
                            name=f"I-waitfix-{cnt[0]}",
                            sync_info=mybir.SyncInfo(on_wait=[w], on_update=[]),
                        )
                        nop.engine = inst.engine
                        out.append(nop)
                    inst.sync_info = mybir.SyncInfo(
                        on_wait=[waits[-1]], on_update=list(si.on_update or [])
                    )
                    n += 1
                out.append(inst)
            bb.instructions = out
    return n


def build_kernel():
    nc = bass.Bass("TRN2", target_bir_lowering=False, debug=False,
                   num_devices=NCORES)

    xT_d = nc.dram_tensor("xT", [B, D, L], F32, kind="ExternalInput").ap()
    whg_d = nc.dram_tensor("whg8", [2, 128, 3, 1536], FP8,
                           kind="ExternalInput").ap()
    wout_d = nc.dram_tensor("wout16", [2, 128, 6, D], BF16,
                            kind="ExternalInput").ap()
    p1_d = nc.dram_tensor("p18", [128, 3, MLPD], FP8, kind="ExternalInput").ap()
    p2_d = nc.dram_tensor("p28", [128, 12, D], FP8, kind="ExternalInput").ap()
    cw_d = nc.dram_tensor("convw8", [DC, 128, 9, 128], FP8,
                          kind="ExternalInput").ap()
    dwb_d = nc.dram_tensor("dwb", [128, DC], F32, kind="ExternalInput").ap()
    p1b_d = nc.dram_tensor("p1b", [128, 12], F32, kind="ExternalInput").ap()
    p2bT_d = nc.dram_tensor("p2bT", [1, D], BF16, kind="ExternalInput").ap()
    ident_d = nc.dram_tensor("ident", [128, 128], BF16,
                             kind="ExternalInput").ap()
    out_d = nc.dram_tensor("outT", [B, D, L], F32, kind="ExternalOutput").ap()

    from contextlib import ExitStack
    with tile.TileContext(nc) as tc, ExitStack() as ctx:
        big = ctx.enter_context(tc.tile_pool(name="big", bufs=1))
        wpool = ctx.enter_context(tc.tile_pool(name="wpool", bufs=1))
        work = ctx.enter_context(tc.tile_pool(name="work", bufs=2))
        psum = ctx.enter_context(tc.tile_pool(name="psum", bufs=1, space="PSUM"))

        # ---------------- persistent weights / constants (loaded once)
        whg_t = [wpool.tile([128, 3, 1536], FP8, tag=f"whg{g}", name=f"whg{g}")
                 for g in range(2)]
        wout_t = [wpool.tile([128, 6, D], BF16, tag=f"wo{g}", name=f"wo{g}")
                  for g in range(2)]
        for g in range(2):
            nc.sync.dma_start(whg_t[g][:], whg_d[g])
            nc.sync.dma_start(wout_t[g][:], wout_d[g])
        p1w = wpool.tile([128, 3, MLPD], FP8, tag="p1w", name="p1w")
        nc.sync.dma_start(p1w[:], p1_d)
        p2w = wpool.tile([128, 12, D], FP8, tag="p2w", name="p2w")
        nc.sync.dma_start(p2w[:], p2_d)
        cw_t = [wpool.tile([128, 9, 128], FP8, tag=f"cw{c}", name=f"cw{c}")
                for c in range(DC)]
        for c in range(DC):
            nc.sync.dma_start(cw_t[c][:], cw_d[c])
        dwb_t = wpool.tile([128, DC], F32, tag="dwb", name="dwb")
        nc.sync.dma_start(dwb_t[:], dwb_d)
        p1b_t = wpool.tile([128, 12], F32, tag="p1b", name="p1b")
        nc.sync.dma_start(p1b_t[:], p1b_d)
        p2bT_t = wpool.tile([1, D], BF16, tag="p2bT", name="p2bT")
        nc.sync.dma_start(p2bT_t[:], p2bT_d)
        ident_t = wpool.tile([128, 128], BF16, tag="ident", name="ident")
        nc.sync.dma_start(ident_t[:], ident_d)

        zsel16 = wpool.tile([128, 129], BF16, tag="zsel16", name="zsel16")
        nc.vector.memset(zsel16[:], 0.0)
        nc.vector.memset(zsel16[:, 64:65], 1.0)
        ones1 = wpool.tile([65, 128], BF16, tag="ones1", name="ones1")
        nc.vector.memset(ones1[:], 1.0)
        onesN = wpool.tile([1, NT], BF16, tag="onesN", name="onesN")
        nc.vector.memset(onesN[:], 1.0)
        eps_t = wpool.tile([128, 1], F32, tag="eps", name="eps")
        nc.vector.memset(eps_t[:], EPS)

        def r32(ap):
            return ap.bitcast(F32R)

        xq = {}

        def load_x(b):
            ts = [big.tile([128, L], BF16, tag=f"x{c}", name=f"x{c}b{b}",
                           bufs=2) for c in range(DC)]
            for c, t in enumerate(ts):
                for h0, hn in ((0, 1792), (1792, 1344)):
                    nc.gpsimd.dma_start(
                        t[:, h0:h0 + hn],
                        xT_d[b, c * 128:(c + 1) * 128, h0:h0 + hn])
            xq[b] = ts

        load_x(0)
        for b in range(B):
            # ---------------- batch-persistent tiles
            x_t = xq[b]
            # padded fp8 activations: 1 guard element + 4 planes of LP
            xh = big.tile([128, 1 + 3 * LP], FP8, tag="act8", name="act8")
            xf = xh[:]
            nc.gpsimd.memset(xh[:, 0:1], 0.0)              # guard
            # zero the pad column (index 56 of each 57-row) of planes 0..2
            pads = bass.AP(tensor=xf.tensor, offset=xf.offset + 1 + NB,
                           ap=[list(xf.ap[0]), [LP, 3], [NBP, NB]])
            nc.gpsimd.memset(pads, 0.0)

            def xh_pl(c):
                return xh[:, 1 + c * LP: 1 + (c + 1) * LP]

            hc = big.tile([128, 3, L], FP8, tag="hc", name="hc")

            # ============================================= layernorm helper
            def norm_stats(src):
                """src(c, sl) -> f32 AP.  Returns [(numu, inv, blks), ...]
                with per-block stats rows at partitions {0,32,64,96}."""
                res = []
                for grp, blks in enumerate(STATS_GROUPS):
                    sA = psum.tile([65, NT], F32, tag="P0", name="sA")
                    sB = psum.tile([65, NT], F32, tag="P1", name="sB")
                    nlast = len(blks) - 1
                    for i, blk in enumerate(blks):
                        sl = slice(blk * NT, (blk + 1) * NT)
                        sq = work.tile([128, NT], BF16, tag="sq", name="sq",
                                       bufs=1)
                        for c in range(DC):
                            nc.scalar.activation(sq[:], src(c, sl), AF.Square)
                            nc.tensor.matmul(
                                sA[:], zsel16[:, 64 - 32 * i:129 - 32 * i],
                                src(c, sl), start=(i == 0 and c == 0),
                                stop=(i == nlast and c == DC - 1),
                                skip_group_check=True)
                            nc.tensor.matmul(
                                sB[:], zsel16[:, 64 - 32 * i:129 - 32 * i],
                                sq[:], start=(i == 0 and c == 0),
                                stop=(i == nlast and c == DC - 1),
                                skip_group_check=True)
                    numu = work.tile([65, NT], BF16, tag=f"numu{grp}",
                                     name=f"numu{grp}", bufs=1)
                    nc.scalar.activation(numu[:], sA[:], AF.Copy,
                                         scale=-1.0 / D)
                    scr = work.tile([65, NT], F32, tag="nscr", name="nscr",
                                    bufs=1)
                    nc.scalar.activation(scr[:], sA[:], AF.Square,
                                         scale=1.0 / D)
                    nc.vector.scalar_tensor_tensor(scr[:], sB[:], 1.0 / D,
                                                   scr[:], ALU.mult,
                                                   ALU.subtract)
                    nc.scalar.activation(scr[:], scr[:], AF.Sqrt,
                                         bias=eps_t[0:65, :])
                    scr2 = work.tile([65, NT], F32, tag="nscr2",
                                     name="nscr2", bufs=1)
                    nc.vector.reciprocal(scr2[:], scr[:])
                    inv = work.tile([65, NT], BF16, tag=f"inv{grp}",
                                    name=f"inv{grp}", bufs=1)
                    nc.vector.tensor_copy(inv[:], scr2[:])
                    res.append((numu, inv, blks))
                return res

            def norm_apply(stats, src, store):
                for numu, inv, blks in stats:
                    for i, blk in enumerate(blks):
                        sl = slice(blk * NT, (blk + 1) * NT)
                        ib = psum.tile([128, NT], F32, tag="P4", name="ib")
                        nc.tensor.matmul(ib[:],
                                         ones1[32 * i:32 * i + 1, :],
                                         inv[32 * i:32 * i + 1, :],
                                         start=True, stop=True)
                        ibs = work.tile([128, NT], BF16, tag="ibs",
                                        name="ibs", bufs=2)
                        nc.scalar.activation(ibs[:], ib[:], AF.Copy)
                        for c in range(DC):
                            apc = psum.tile([128, NT], F32,
                                            tag=f"P{2 + c % 2}", name="apc")
                            nc.tensor.matmul(apc[:],
                                             ones1[32 * i:32 * i + 1, :],
                                             numu[32 * i:32 * i + 1, :],
                                             start=True, stop=False,
                                             skip_group_check=True)
                            nc.tensor.matmul(apc[:], ident_t[:],
                                             src(c, sl),
                                             start=False, stop=True,
                                             skip_group_check=True)
                            store(c, blk, apc, ibs)

            # ------------------------------------------------ norm1 -> xh
            st1 = norm_stats(lambda c, sl: x_t[c][:, sl])

            def store_xh(c, blk, apc, ib):
                base = xh_pl(c)
                dst = bass.AP(tensor=base.tensor,
                              offset=base.offset + blk * 8 * NBP,
                              ap=[list(base.ap[0]), [NBP, 8], [1, NB]])
                src3 = apc[:].rearrange("p (rr cc) -> p rr cc", cc=NB)
                ib3 = ib[:].rearrange("p (rr cc) -> p rr cc", cc=NB)
                nc.vector.tensor_tensor(dst, src3, ib3, ALU.mult)

            norm_apply(st1, lambda c, sl: x_t[c][:, sl], store_xh)
            if b + 1 < B:
                load_x(b + 1)

            # ============================================= depthwise conv 3x3
            # cw slot order: [t4, t0,t2, t3,t5, t6,t8, t1,t7]
            for c in range(DC):
                base = xh_pl(c)
                pstride = list(base.ap[0])

                def mv(off, d1, n):
                    return bass.AP(tensor=base.tensor,
                                   offset=base.offset + off,
                                   ap=[pstride, [d1, 2], [1, n]])

                for slab in range(NBLK):
                    cp = psum.tile([128, NTP], F32,
                                   tag=("P6" if slab % 2 == 0 else "P7"),
                                   name="cp")
                    r0 = 8 * slab
                    # solo center tap (0,0): full padded range, starts group
                    for o0, o1 in ((0, 228), (228, NTP)):
                        nc.tensor.matmul(cp[:, o0:o1], cw_t[c][:, 0, :],
                                         base[:, r0 * NBP + o0:r0 * NBP + o1],
                                         start=(o0 == 0), stop=False,
                                         skip_group_check=True)
                    # column pairs (dr,-1)&(dr,+1): dim1 stride 2
                    for dr, slot in ((-1, 1), (0, 3), (1, 5)):
                        lo = max(0, -(r0 + dr))
                        hi = max(0, (r0 + 8 + dr) - NB)
                        orow0, orow1 = lo, 8 - hi
                        ncols = (orow1 - orow0) * NBP - 1
                        src0 = (r0 + orow0 + dr) * NBP - 1
                        mid = ncols // 2
                        for c0, c1 in ((0, mid), (mid, ncols)):
                            nc.tensor.matmul(
                                cp[:, orow0 * NBP + c0:orow0 * NBP + c1],
                                cw_t[c][:, slot:slot + 2, :],
                                mv(src0 + c0, 2, c1 - c0),
                                start=False, stop=False, perf_mode=DRM,
                                skip_group_check=True)
                    # row pair (-1,0)&(+1,0): dim1 stride 2*NBP
                    lo = max(0, 1 - r0)
                    hi = max(0, (r0 + 8) - (NB - 1))
                    orow0, orow1 = lo, 8 - hi
                    ncols = (orow1 - orow0) * NBP - 1
                    src0 = (r0 + orow0 - 1) * NBP
                    mid = ncols // 2
                    interior = slab not in (0, NBLK - 1)
                    for si, (c0, c1) in enumerate(((0, mid), (mid, ncols))):
                        nc.tensor.matmul(
                            cp[:, orow0 * NBP + c0:orow0 * NBP + c1],
                            cw_t[c][:, 7:9, :], mv(src0 + c0, 2 * NBP, c1 - c0),
                            start=False, stop=(interior and si == 1),
                            perf_mode=DRM, skip_group_check=True)
                    # slab edge rows 0 / 55 get only one of the (+-1,0) taps
                    if slab == 0:
                        nc.tensor.matmul(cp[:, 0:NB], cw_t[c][:, 8, :],
                                         base[:, NBP:NBP + NB],
                                         start=False, stop=True,
                                         skip_group_check=True)
                    if slab == NBLK - 1:
                        nc.tensor.matmul(cp[:, 7 * NBP:7 * NBP + NB],
                                         cw_t[c][:, 7, :],
                                         base[:, 54 * NBP:54 * NBP + NB],
                                         start=False, stop=True,
                                         skip_group_check=True)
                    # readout: drop pad columns, descale, +bias -> fp8
                    cin = cp[:].rearrange("p (rr cc) -> p rr cc",
                                          cc=NBP)[:, :, 0:NB]
                    hout = hc[:, c, slab * NT:(slab + 1) * NT].rearrange(
                        "p (rr cc) -> p rr cc", cc=NB)
                    nc.scalar.activation(hout, cin, AF.Identity,
                                         scale=1.0 / CONVS,
                                         bias=dwb_t[:, c:c + 1])

            # ============================================= bidirectional GRU
            hs_t = [big.tile([128, 6, L], BF16, tag=f"hs{g}", name=f"hs{g}")
                    for g in range(2)]
            pending_scan = [None]

            def flush_scan():
                if pending_scan[0] is not None:
                    for emit in pending_scan[0]:
                        emit()
                    pending_scan[0] = None

            for j in range(6):
                for g in range(2):
                    halves = HALVES if g == 0 else HALVES[::-1]
                    for h0, hn, blks in halves:
                        a_t = work.tile([128, 1792], BF16, tag="a_t",
                                        name="a_t", bufs=2)
                        s_t = work.tile([128, 1792], BF16, tag="s_t",
                                        name="s_t", bufs=2)
                        border = blks if g == 0 else blks[::-1]
                        for blk in border:
                            lo = (blk - blks[0]) * NT
                            hp = psum.tile([128, NT], F32,
                                           tag=f"P{2 + blk % 2}", name="hp")
                            gp = psum.tile([128, NT], F32,
                                           tag=f"P{4 + blk % 2}", name="gp")
                            for s2 in (0, 1):
                                t0 = blk * NT + s2 * SUB
                                rhs = hc[:, 0:2, t0:t0 + SUB]
                                nc.tensor.matmul(
                                    hp[:, s2 * SUB:(s2 + 1) * SUB],
                                    whg_t[g][:, 0:2, j * 128:(j + 1) * 128],
                                    rhs, start=(s2 == 0), stop=False,
                                    perf_mode=DRM, skip_group_check=True)
                                nc.tensor.matmul(
                                    gp[:, s2 * SUB:(s2 + 1) * SUB],
                                    whg_t[g][:, 0:2,
                                             DI + j * 128:DI + (j + 1) * 128],
                                    rhs, start=(s2 == 0), stop=False,
                                    perf_mode=DRM, skip_group_check=True)
                            sl448 = slice(blk * NT, (blk + 1) * NT)
                            nc.tensor.matmul(
                                hp[:], whg_t[g][:, 2, j * 128:(j + 1) * 128],
                                hc[:, 2, sl448], start=False, stop=True,
                                skip_group_check=True)
                            nc.tensor.matmul(
                                gp[:],
                                whg_t[g][:, 2, DI + j * 128:DI + (j + 1) * 128],
                                hc[:, 2, sl448], start=False, stop=True,
                                skip_group_check=True)
                            nc.scalar.activation(s_t[:, lo:lo + NT], hp[:],
                                                 AF.Sigmoid)
                            nc.scalar.activation(a_t[:, lo:lo + NT], gp[:],
                                                 AF.Sigmoid)
                            # g = max(h + 0.5, sigmoid(h)), in place on s
                            nc.vector.scalar_tensor_tensor(
                                s_t[:, lo:lo + NT], hp[:], 0.5,
                                s_t[:, lo:lo + NT], ALU.add, ALU.max)
                        # b' = (a-1)*g = -b; wout is negated host-side
                        t_t = work.tile([128, 1792], BF16, tag="t_t",
                                        name="t_t", bufs=2)
                        nc.vector.scalar_tensor_tensor(
                            t_t[:, 0:hn], a_t[:, 0:hn], 1.0, s_t[:, 0:hn],
                            ALU.subtract, ALU.mult)

                        def make_scan(g=g, j=j, h0=h0, hn=hn, a_t=a_t,
                                      t_t=t_t, first=(h0 == 0) == (g == 0)):
                            eng = nc.vector
                            def emit():
                                if g == 0:
                                    init = (0.0 if h0 == 0
                                            else hs_t[0][:, j, 1791:1792])
                                    eng.tensor_tensor_scan(
                                        hs_t[0][:, j, h0:h0 + hn],
                                        a_t[:, 0:hn], t_t[:, 0:hn], init,
                                        ALU.mult, ALU.add)
                                else:
                                    rv = slice(None, None, -1)
                                    init = (0.0 if h0 == 1792
                                            else hs_t[1][:, j, 1792:1793])
                                    eng.tensor_tensor_scan(
                                        hs_t[1][:, j, h0:h0 + hn][:, rv],
                                        a_t[:, 0:hn][:, rv],
                                        t_t[:, 0:hn][:, rv], init,
                                        ALU.mult, ALU.add)
                            return emit

                        if (h0 == 0) == (g == 0):
                            flush_scan()          # flush prev (j,g) pair
                            pending_scan[0] = [make_scan()]
                        else:
                            pending_scan[0].append(make_scan())
            flush_scan()

            # out-projection (bf16) + residual accumulate, in place into x_t
            for blk in range(NBLK):
                sl = slice(blk * NT, (blk + 1) * NT)
                for dc in range(DC):
                    y_ps = psum.tile([128, NT], F32,
                                     tag=f"P{6 + (blk * DC + dc) % 2}",
                                     name="y_ps")
                    for g in range(2):
                        for j in range(6):
                            nc.tensor.matmul(
                                y_ps[:],
                                wout_t[g][:, j, dc * 128:(dc + 1) * 128],
                                hs_t[g][:, j, sl],
                                start=(g == 0 and j == 0),
                                stop=(g == 1 and j == 5))
                    nc.vector.tensor_tensor(x_t[dc][:, sl], y_ps[:],
                                            x_t[dc][:, sl], ALU.add)

            # ============================================= norm2 -> yh
            st2 = norm_stats(lambda c, sl: x_t[c][:, sl])

            def yh_pl(c):
                return xh[:, 1 + c * LP: 1 + c * LP + L]

            def store_yh(c, blk, apc, ib):
                sl = slice(blk * NT, (blk + 1) * NT)
                nc.vector.tensor_tensor(yh_pl(c)[:, sl], apc[:], ib[:],
                                        ALU.mult)

            norm_apply(st2, lambda c, sl: x_t[c][:, sl], store_yh)

            def yh_rhs(kk, t0, n):
                return bass.AP(tensor=xf.tensor,
                               offset=xf.offset + 1 + kk * LP + t0,
                               ap=[list(xf.ap[0]), [LP, 2], [1, n]])

            # ============================================= MLP (fp8 DR)
            for blk in range(NBLK):
                sl = slice(blk * NT, (blk + 1) * NT)
                o_ps = [psum.tile([128, NT], F32, tag=f"P{5 + dc}",
                                  name=f"o{dc}") for dc in range(DC)]
                for w in range(3):
                    qt = work.tile([128, 4, NT], FP8, tag="qt", name="qt",
                                   bufs=2)
                    for mi in range(4):
                        mc = 4 * w + mi
                        q_ps = psum.tile([128, NT], F32, tag=f"P{mi}",
                                         name="q_ps")
                        for s2 in (0, 1):
                            nc.tensor.matmul(
                                q_ps[:, s2 * SUB:(s2 + 1) * SUB],
                                p1w[:, 0:2, mc * 128:(mc + 1) * 128],
                                yh_rhs(0, blk * NT + s2 * SUB, SUB),
                                start=(s2 == 0), stop=False,
                                perf_mode=DRM, skip_group_check=True)
                        nc.tensor.matmul(
                            q_ps[:], p1w[:, 2, mc * 128:(mc + 1) * 128],
                            yh_pl(2)[:, sl], start=False, stop=True,
                            skip_group_check=True)
                        nc.scalar.activation(qt[:, mi, :], q_ps[:], AF.Gelu,
                                             scale=1.0 / WS,
                                             bias=p1b_t[:, mc:mc + 1])
                    for dc in range(DC):
                        for mp in (0, 2):
                            for s2 in (0, 1):
                                nc.tensor.matmul(
                                    o_ps[dc][:, s2 * SUB:(s2 + 1) * SUB],
                                    p2w[:, 4 * w + mp:4 * w + mp + 2,
                                        dc * 128:(dc + 1) * 128],
                                    qt[:, mp:mp + 2, s2 * SUB:(s2 + 1) * SUB],
                                    start=(w == 0 and mp == 0 and s2 == 0),
                                    stop=False,
                                    perf_mode=DRM, skip_group_check=True)
                oo = []
                for dc in range(DC):
                    # + p2b (pre-scaled by WS) via K=1 matmul
                    nc.tensor.matmul(o_ps[dc][:],
                                     p2bT_t[:, dc * 128:(dc + 1) * 128],
                                     onesN[:], start=False, stop=True,
                                     skip_group_check=True)
                    ot = work.tile([128, NT], F32, tag="oo", name="oo", bufs=2)
                    nc.vector.scalar_tensor_tensor(ot[:], o_ps[dc][:],
                                                   1.0 / WS, x_t[dc][:, sl],
                                                   ALU.mult, ALU.add)
                    oo.append(ot)
                for dc in range(DC):
                    nc.sync.dma_start(out_d[b, dc * 128:(dc + 1) * 128, sl],
                                      oo[dc][:])

    return nc


# ------------------------------------------------------------------ host side
_NC_CACHE = {}


def _get_nc():
    if "nc" not in _NC_CACHE:
        nc = build_kernel()
        _fix_multiwaits(nc)
        _NC_CACHE["nc"] = nc
    return _NC_CACHE["nc"]


def _q8(a, scale=1.0):
    return np.clip(np.asarray(a, np.float32) * scale, -240.0,
                   240.0).astype(ml_dtypes.float8_e4m3)


def _prep_weights(inp):
    f = np.float32
    norm_w = np.asarray(inp["norm_w"], f)
    norm_b = np.asarray(inp["norm_b"], f)
    dw_w = np.asarray(inp["dw_w"], f)[:, 0]              # [D,3,3]
    dw_wf = dw_w * norm_w[:, None, None]
    dw_bf = np.asarray(inp["dw_b"], f) + norm_b * dw_w.sum(axis=(1, 2))
    p1_w = np.asarray(inp["p1_w"], f)
    p1f = p1_w * np.asarray(inp["norm2_w"], f)[:, None]
    p1bf = np.asarray(inp["p1_b"], f) + np.asarray(inp["norm2_b"], f) @ p1_w

    # GRU in-proj: [D, 2DI] -> [128, 3, 1536], gate half negated
    whg8 = np.zeros((2, 128, 3, 1536), ml_dtypes.float8_e4m3)
    for g, key in enumerate(["gru1_whg", "gru2_whg"]):
        w = np.asarray(inp[key], f).copy()               # [384, 1536]
        w[:, DI:] = -w[:, DI:]
        for k in range(3):
            whg8[g, :, k, :] = _q8(w[k * 128:(k + 1) * 128, :])
    # GRU out-proj bf16, NEGATED (the scan produces -h): [DI, D] -> [128,6,D]
    wout16 = np.zeros((2, 128, 6, D), ml_dtypes.bfloat16)
    for g, key in enumerate(["gru1_wout", "gru2_wout"]):
        w = -np.asarray(inp[key], f)
        for k in range(6):
            wout16[g, :, k, :] = w[k * 128:(k + 1) * 128, :].astype(
                ml_dtypes.bfloat16)
    # MLP
    p18 = np.zeros((128, 3, MLPD), ml_dtypes.float8_e4m3)
    for k in range(3):
        p18[:, k, :] = _q8(p1f[k * 128:(k + 1) * 128, :], WS)
    p28 = np.zeros((128, 12, D), ml_dtypes.float8_e4m3)
    p2 = np.asarray(inp["p2_w"], f)
    for k in range(12):
        p28[:, k, :] = _q8(p2[k * 128:(k + 1) * 128, :], WS)
    # conv diag blocks, slot order [4, 0,2, 3,5, 6,8, 1,7]
    slots = [4, 0, 2, 3, 5, 6, 8, 1, 7]
    cw8 = np.zeros((DC, 128, 9, 128), ml_dtypes.float8_e4m3)
    ar = np.arange(128)
    for c in range(DC):
        for si, tap in enumerate(slots):
            dr, dcc = tap // 3, tap % 3
            cw8[c, ar, si, ar] = _q8(
                dw_wf[c * 128:(c + 1) * 128, dr, dcc], CONVS)

    return dict(
        whg8=whg8, wout16=wout16, p18=p18, p28=p28, convw8=cw8,
        dwb=np.ascontiguousarray(dw_bf.reshape(DC, 128).T, f),
        p1b=np.ascontiguousarray(p1bf.reshape(12, 128).T, f),
        p2bT=np.ascontiguousarray(
            (np.asarray(inp["p2_b"], f) * WS).reshape(1, D)).astype(
                ml_dtypes.bfloat16),
        ident=np.eye(128, dtype=f).astype(ml_dtypes.bfloat16),
    )


def kernel(**inputs):
    x = np.asarray(inputs["x"], np.float32)              # [16, L, D]
    w = _prep_weights(inputs)
    nc = _get_nc()

    in_maps = []
    for core in range(NCORES):
        xb = x[core * B:(core + 1) * B]                  # [B, L, D]
        m = dict(w)
        m["xT"] = np.ascontiguousarray(xb.transpose(0, 2, 1))
        in_maps.append(m)

    res = run_bass_kernel_spmd(nc, in_maps, core_ids=list(range(NCORES)))
    outs = []
    for core in range(NCORES):
        oT = res.results[core]["outT"]                   # [B, D, L]
        outs.append(oT.transpose(0, 2, 1))
    return np.ascontiguousarray(np.concatenate(outs, axis=0), np.float32)
